# revision 14
# baseline (speedup 1.0000x reference)
"""Bass/Trainium2 kernel for nn_BilinearPairedLayer.

Math (per batch b):
  xl = concat([x, shift_down(x,1), shift_up(x,1)], -1)      # [N, 192]
  xr = concat([x, shift_up(x,1), shift_down(x,1)], -1)
  hl = relu(xl @ W_l.T + b_l)                               # [N, 128]
  hr = relu(xr @ W_r.T + b_r)
  out[i,j,k] = sum_g (hl @ W_bil[k])[i,g] * hr[j,g] + b_bil[k]   # [N, N, 2]

Sharding: data-parallel over B — core c computes batch b=c (B=8, 8 cores).

The kernel is output-DMA-bound: 8 MiB of fp32 output per core at the
~410 GB/s per-core HBM write rate is a ~20.5 us floor. Design:
  - ALL matmuls in bf16 (operands cast host-side / on PSUM->SBUF copy;
    fp32 PSUM accumulation). bf16 streams 1 col/cycle at 2.4 GHz vs
    ~2.7 cycles/col for fp32r, so PE production (~12-14 us) sits well
    under the DMA floor. Measured end-to-end rel err ~4e-3 (gate 2e-2).
  - context shifts are free: shifted feature chunks of xl^T are column
    offsets into xT thanks to 2 zero guard columns on each side
  - prologue is minimized so the first 512 KiB output DMA starts ~4 us
    in: hl cols 0:128 -> t cols 0:128 -> hr cols 0:512 -> out tile
    (0,0); a short bf16 warmup spinner burns the PE cold-clock window
    while the x DMA is in flight
  - out tile (iblk, jh): PSUM po_k = tT_k[:, iblk].T @ hrT[:, jh*512];
    b_bil + (j,k)-interleave fused into PSUM->SBUF copies (DVE
    tensor_scalar_add writes k=0 stride-2 columns, ACT Identity+bias
    writes k=1), then one 512 KiB DMA per tile
  - out DMAs alternate between the sync (HWDGE) and gpsimd (SWDGE)
    queues so both stay fed; ACT/DVE stay off the DMA-issue path. The
    last tile's DMA is quartered to shorten the tail receipt.
"""

import numpy as np

B, N, NIN = 8, 1024, 64
H = 128
NOUT = 2
NCH = 512  # out-tile free-dim chunk (one PSUM bank of fp32)
GD = 2     # zero guard columns on each side of xT
NWARM = 4

_cached = {}


def _build():
    import concourse.bacc as bacc
    import concourse.mybir as mybir
    import concourse.tile as tile

    f32 = mybir.dt.float32
    bf16 = mybir.dt.bfloat16
    AF = mybir.ActivationFunctionType
    ALU = mybir.AluOpType

    nc = bacc.Bacc("TRN2", target_bir_lowering=False, debug=False, num_devices=8)

    xt_d = nc.dram_tensor("x_t", [NIN, N + 2 * GD], bf16, kind="ExternalInput").ap()
    wlt_d = nc.dram_tensor("w_lt", [NIN, 3, H], bf16, kind="ExternalInput").ap()
    bl_d = nc.dram_tensor("b_l", [H], f32, kind="ExternalInput").ap()
    wrt_d = nc.dram_tensor("w_rt", [NIN, 3, H], bf16, kind="ExternalInput").ap()
    br_d = nc.dram_tensor("b_r", [H], f32, kind="ExternalInput").ap()
    wb_d = nc.dram_tensor("w_bil", [NOUT, H, H], bf16, kind="ExternalInput").ap()
    bb_d = nc.dram_tensor("b_bil", [NOUT], f32, kind="ExternalInput").ap()
    out_d = nc.dram_tensor("out", [N, N, NOUT], f32, kind="ExternalOutput").ap()
    # row-block view: [8 blocks, 128 rows, (j,k) interleaved 2048]
    out_v = out_d.rearrange("(t p) n k -> t p (n k)", p=128)

    with tile.TileContext(nc) as tc:
        with (
            tc.tile_pool(name="const", bufs=1) as const,
            tc.tile_pool(name="ps", bufs=7, space="PSUM") as ps,
            tc.tile_pool(name="wps", bufs=1, space="PSUM") as wps_pool,
            tc.tile_pool(name="ob", bufs=16) as ob,
        ):
            # ---- input DMAs. x on the sync queue (HWDGE), split so the
            # first piece (enough for the j0=0 h-chunks) lands earliest;
            # weights/biases on the gpsimd queue in first-use order.
            XSPLIT = NCH + 2 * GD
            xT = const.tile([NIN, N + 2 * GD], bf16)
            nc.sync.dma_start(out=xT[:, 0:XSPLIT], in_=xt_d[:, 0:XSPLIT])
            nc.sync.dma_start(out=xT[:, XSPLIT:], in_=xt_d[:, XSPLIT:])
            wlT = const.tile([NIN, 3, H], bf16)
            nc.gpsimd.dma_start(out=wlT, in_=wlt_d)
            bl_s = const.tile([H, 1], f32)
            nc.gpsimd.dma_start(out=bl_s, in_=bl_d.unsqueeze(1))
            wrT = const.tile([NIN, 3, H], bf16)
            nc.gpsimd.dma_start(out=wrT, in_=wrt_d)
            br_s = const.tile([H, 1], f32)
            nc.gpsimd.dma_start(out=br_s, in_=br_d.unsqueeze(1))
            wb0 = const.tile([H, H], bf16)
            nc.gpsimd.dma_start(out=wb0, in_=wb_d[0])
            wb1 = const.tile([H, H], bf16)
            nc.gpsimd.dma_start(out=wb1, in_=wb_d[1])
            bb_s = const.tile([128, NOUT], f32)
            nc.gpsimd.dma_start(
                out=bb_s, in_=bb_d.unsqueeze(0).broadcast_to([128, NOUT])
            )

            # ---- PE warmup spinner: burn the HAM cold-clock window while
            # the x DMA is in flight. Also pre-pull the lazy ACT tables.
            # Plain-fp32 matmuls register as PE-busy for the HAM clock gate
            # (bf16 alone lets the clock idle-gate back to 1.2 GHz), so
            # fp32 heartbeats are woven through the whole kernel.
            warm = const.tile([128, 256], f32)
            nc.vector.memset(warm, 0.0)
            actscratch = const.tile([1, 4], f32)
            nc.scalar.activation(actscratch[0:1, 0:2], warm[0:1, 0:2], AF.Relu)
            nc.scalar.activation(actscratch[0:1, 2:4], warm[0:1, 0:2], AF.Identity)
            wps = wps_pool.tile([128, NCH], f32, tag="warm")

            def warmmm():
                nc.tensor.matmul(
                    wps[:, 0:256], warm[:, 0:128], warm,
                    start=True, stop=True, skip_group_check=True,
                )

            warm_bf = const.tile([128, 128], bf16)
            nc.vector.memset(warm_bf, 0.0)

            def heartbeat():
                # single-instruction bf16 matmul: keeps the HAM activity
                # window fed through PE dependency gaps without the 2x
                # instruction cost of a plain-fp32 matmul
                nc.tensor.matmul(
                    wps[:, 0:128], warm_bf, warm_bf,
                    start=True, stop=True, skip_group_check=True,
                )

            for _ in range(NWARM):
                warmmm()

            hlT = const.tile([H, N], bf16)
            hrT = const.tile([H, N], bf16)
            tT0 = const.tile([H, N], bf16)
            tT1 = const.tile([H, N], bf16)

            def h_piece(dst, wt, bias, s1, j0, w, on_act=True, split=False):
                # chunk 1 is shift_down (src col i-1) for xl, shift_up (i+1) for xr
                ph = ps.tile([128, w], f32, tag="ps")
                for c, s in ((0, 0), (1, s1), (2, -s1)):
                    nc.tensor.matmul(
                        ph[:, :],
                        wt[:, c, :],
                        xT[:, GD + j0 + s : GD + j0 + s + w],
                        start=(c == 0), stop=(c == 2),
                    )
                halves = ((0, w // 2), (w // 2, w)) if split else ((0, w),)
                for lo, hi in halves:
                    if on_act:
                        nc.scalar.activation(
                            dst[:, j0 + lo : j0 + hi], ph[:, lo:hi], AF.Relu,
                            bias=bias[:, 0:1], scale=1.0,
                        )
                    else:
                        nc.vector.tensor_scalar(
                            dst[:, j0 + lo : j0 + hi], ph[:, lo:hi], bias[:, 0:1],
                            0.0, ALU.add, ALU.max,
                        )

            def t_piece(j0, w):
                for wb, tT in ((wb0, tT0), (wb1, tT1)):
                    pt = ps.tile([128, w], f32, tag="ps")
                    nc.tensor.matmul(pt[:, :], wb, hlT[:, j0 : j0 + w],
                                     start=True, stop=True)
                    nc.vector.tensor_copy(tT[:, j0 : j0 + w], pt)

            _dmaq = [0]

            def out_tile(iblk, j0, w=NCH, last=False, hb=True):
                if hb:
                    heartbeat()
                ohalf = ob.tile([128, 2 * w], f32, tag="ob")
                for k, tT in ((0, tT0), (1, tT1)):
                    po = ps.tile([128, w], f32, tag="ps")
                    nc.tensor.matmul(
                        po[:, :],
                        tT[:, iblk * 128 : (iblk + 1) * 128],
                        hrT[:, j0 : j0 + w],
                        start=True, stop=True,
                    )
                    dst = ohalf[:, k : 2 * w : 2]
                    if k == 0:
                        nc.vector.tensor_scalar_add(dst, po[:, :], bb_s[:, 0:1])
                    else:
                        nc.scalar.activation(
                            dst, po[:, :], AF.Identity, bias=bb_s[:, 1:2], scale=1.0
                        )
                _dmaq[0] += 1
                eng = nc.sync if _dmaq[0] % 2 == 1 else nc.gpsimd
                dst_v = out_v[iblk][:, 2 * j0 : 2 * j0 + 2 * w]
                if last:
                    # quarter the final DMA so the tail receipt is short
                    q = w // 2
                    for qi in range(4):
                        e = nc.sync if qi % 2 == 0 else nc.gpsimd
                        e.dma_start(
                            out=dst_v[:, qi * q : (qi + 1) * q],
                            in_=ohalf[:, qi * q : (qi + 1) * q],
                        )
                else:
                    eng.dma_start(out=dst_v, in_=ohalf)

            # emission order: the whole h/t stage runs up front as one
            # dense PE stream (~16 matmuls), then the 16 out tiles form a
            # uniform production pipeline — per tile the PE does 2 matmuls
            # + 1 heartbeat (~1.0us) against the 1.28us DMA drain period,
            # so the DMA queues never starve and the PE never idles long
            # enough to re-throttle the clock. The first row block is
            # split into 256-col tiles to shorten the first DMA's path.
            h_piece(hlT, wlT, bl_s, -1, 0, NCH, on_act=False)
            h_piece(hlT, wlT, bl_s, -1, NCH, NCH, on_act=False)
            h_piece(hrT, wrT, br_s, +1, 0, NCH, split=True)
            h_piece(hrT, wrT, br_s, +1, NCH, NCH)
            t_piece(0, NCH)
            t_piece(NCH, NCH)
            out_tile(0, 0, 256)
            out_tile(0, 256, 256)
            for iblk in range(1, 8):
                out_tile(iblk, 0)
            for iblk in range(0, 4):
                out_tile(iblk, NCH)
            for iblk in range(4, 8):
                out_tile(iblk, NCH, hb=False, last=(iblk == 7))

    nc.finalize()
    return nc


def make_in_maps(x_l, W_l, b_l, W_r, b_r, W_bil, b_bil):
    import ml_dtypes

    bf16 = ml_dtypes.bfloat16

    # host-side layout: W chunks to lhsT [f=64, chunk, h], x to [64, N] with
    # zero guard columns; everything the PE touches is cast to bf16
    def w_chunks(W):
        return np.ascontiguousarray(
            np.asarray(W, np.float32).reshape(H, 3, NIN).transpose(2, 1, 0)
        ).astype(bf16)

    x_l = np.asarray(x_l, np.float32)
    xt = np.zeros((B, NIN, N + 2 * GD), np.float32)
    xt[:, :, GD : GD + N] = x_l.transpose(0, 2, 1)
    xt = xt.astype(bf16)

    com = {
        "w_lt": w_chunks(W_l),
        "b_l": np.ascontiguousarray(b_l, np.float32),
        "w_rt": w_chunks(W_r),
        "b_r": np.ascontiguousarray(b_r, np.float32),
        "w_bil": np.ascontiguousarray(np.asarray(W_bil, np.float32)).astype(bf16),
        "b_bil": np.ascontiguousarray(b_bil, np.float32),
    }
    return [{"x_t": np.ascontiguousarray(xt[c]), **com} for c in range(B)]


def kernel(x_l, W_l, b_l, W_r, b_r, W_bil, b_bil):
    from concourse import bass_utils

    if "nc" not in _cached:
        _cached["nc"] = _build()
    nc = _cached["nc"]

    in_maps = make_in_maps(x_l, W_l, b_l, W_r, b_r, W_bil, b_bil)
    res = bass_utils.run_bass_kernel_spmd(nc, in_maps, core_ids=list(range(B)))
    return np.stack([res.results[c]["out"] for c in range(B)], axis=0)


# revision 15
# speedup vs baseline: 1.1675x; 1.1675x over previous
"""Bass/Trainium2 kernel for nn_BilinearPairedLayer.

Math (per batch b):
  xl = concat([x, shift_down(x,1), shift_up(x,1)], -1)      # [N, 192]
  xr = concat([x, shift_up(x,1), shift_down(x,1)], -1)
  hl = relu(xl @ W_l.T + b_l)                               # [N, 128]
  hr = relu(xr @ W_r.T + b_r)
  out[i,j,k] = sum_g (hl @ W_bil[k])[i,g] * hr[j,g] + b_bil[k]   # [N, N, 2]

Sharding: data-parallel over B — core c computes batch b=c (B=8, 8 cores).

The kernel is output-DMA-bound: 8 MiB of fp32 output per core at the
~410 GB/s per-core HBM write rate is a ~20.5 us floor. Design:
  - ALL matmuls in bf16 (operands cast host-side / on PSUM->SBUF copy;
    fp32 PSUM accumulation). bf16 streams 1 col/cycle at 2.4 GHz vs
    ~2.7 cycles/col for fp32r, so PE production (~12-14 us) sits well
    under the DMA floor. Measured end-to-end rel err ~4e-3 (gate 2e-2).
  - context shifts are free: shifted feature chunks of xl^T are column
    offsets into xT thanks to 2 zero guard columns on each side
  - prologue is minimized so the first 512 KiB output DMA starts ~4 us
    in: hl cols 0:128 -> t cols 0:128 -> hr cols 0:512 -> out tile
    (0,0); a short bf16 warmup spinner burns the PE cold-clock window
    while the x DMA is in flight
  - out tile (iblk, jh): PSUM po_k = tT_k[:, iblk].T @ hrT[:, jh*512];
    b_bil + (j,k)-interleave fused into PSUM->SBUF copies (DVE
    tensor_scalar_add writes k=0 stride-2 columns, ACT Identity+bias
    writes k=1), then one 512 KiB DMA per tile
  - out DMAs alternate between the sync (HWDGE) and gpsimd (SWDGE)
    queues so both stay fed; ACT/DVE stay off the DMA-issue path. The
    last tile's DMA is quartered to shorten the tail receipt.
"""

import numpy as np

B, N, NIN = 8, 1024, 64
H = 128
NOUT = 2
NCH = 512  # out-tile free-dim chunk (one PSUM bank of fp32)
GD = 2     # zero guard columns on each side of xT
NWARM = 6

_cached = {}


def _build():
    import concourse.bacc as bacc
    import concourse.mybir as mybir
    import concourse.tile as tile

    f32 = mybir.dt.float32
    bf16 = mybir.dt.bfloat16
    AF = mybir.ActivationFunctionType
    ALU = mybir.AluOpType

    nc = bacc.Bacc("TRN2", target_bir_lowering=False, debug=False, num_devices=8)

    xt_d = nc.dram_tensor("x_t", [NIN, N + 2 * GD], bf16, kind="ExternalInput").ap()
    wlt_d = nc.dram_tensor("w_lt", [NIN, 3, H], bf16, kind="ExternalInput").ap()
    bl_d = nc.dram_tensor("b_l", [H], f32, kind="ExternalInput").ap()
    wrt_d = nc.dram_tensor("w_rt", [NIN, 3, H], bf16, kind="ExternalInput").ap()
    br_d = nc.dram_tensor("b_r", [H], f32, kind="ExternalInput").ap()
    wb_d = nc.dram_tensor("w_bil", [NOUT, H, H], bf16, kind="ExternalInput").ap()
    bb_d = nc.dram_tensor("b_bil", [NOUT], f32, kind="ExternalInput").ap()
    out_d = nc.dram_tensor("out", [N, N, NOUT], f32, kind="ExternalOutput").ap()
    # row-block view: [8 blocks, 128 rows, (j,k) interleaved 2048]
    out_v = out_d.rearrange("(t p) n k -> t p (n k)", p=128)

    with tile.TileContext(nc) as tc:
        with (
            tc.tile_pool(name="const", bufs=1) as const,
            tc.tile_pool(name="ps", bufs=7, space="PSUM") as ps,
            tc.tile_pool(name="wps", bufs=1, space="PSUM") as wps_pool,
            tc.tile_pool(name="ob", bufs=16) as ob,
        ):
            # ---- input DMAs. x on the sync queue (HWDGE), split so the
            # first piece (enough for the j0=0 h-chunks) lands earliest;
            # weights/biases on the gpsimd queue in first-use order.
            XSPLIT = NCH + 2 * GD
            xT = const.tile([NIN, N + 2 * GD], bf16)
            nc.sync.dma_start(out=xT[:, 0:XSPLIT], in_=xt_d[:, 0:XSPLIT])
            nc.sync.dma_start(out=xT[:, XSPLIT:], in_=xt_d[:, XSPLIT:])
            wlT = const.tile([NIN, 3, H], bf16)
            nc.gpsimd.dma_start(out=wlT, in_=wlt_d)
            bl_s = const.tile([H, 1], f32)
            nc.gpsimd.dma_start(out=bl_s, in_=bl_d.unsqueeze(1))
            wrT = const.tile([NIN, 3, H], bf16)
            nc.gpsimd.dma_start(out=wrT, in_=wrt_d)
            br_s = const.tile([H, 1], f32)
            nc.gpsimd.dma_start(out=br_s, in_=br_d.unsqueeze(1))
            wb0 = const.tile([H, H], bf16)
            nc.gpsimd.dma_start(out=wb0, in_=wb_d[0])
            wb1 = const.tile([H, H], bf16)
            nc.gpsimd.dma_start(out=wb1, in_=wb_d[1])
            bb_s = const.tile([128, NOUT], f32)
            nc.gpsimd.dma_start(
                out=bb_s, in_=bb_d.unsqueeze(0).broadcast_to([128, NOUT])
            )

            # ---- PE warmup spinner: burn the HAM cold-clock window while
            # the x DMA is in flight. Also pre-pull the lazy ACT tables.
            # Plain-fp32 matmuls register as PE-busy for the HAM clock gate
            # (bf16 alone lets the clock idle-gate back to 1.2 GHz), so
            # fp32 heartbeats are woven through the whole kernel.
            warm = const.tile([128, 256], f32)
            nc.vector.memset(warm, 0.0)
            actscratch = const.tile([1, 4], f32)
            nc.scalar.activation(actscratch[0:1, 0:2], warm[0:1, 0:2], AF.Relu)
            nc.scalar.activation(actscratch[0:1, 2:4], warm[0:1, 0:2], AF.Identity)
            wps = wps_pool.tile([128, NCH], f32, tag="warm")

            def warmmm():
                nc.tensor.matmul(
                    wps[:, 0:256], warm[:, 0:128], warm,
                    start=True, stop=True, skip_group_check=True,
                )

            def heartbeat():
                # plain-fp32 matmul: streams each column twice, keeping the
                # PE array's streaming duty high enough that the HAM clock
                # gate holds K=8 through the production phase
                nc.tensor.matmul(
                    wps[:, 0:128], warm[:, 0:128], warm[:, 0:128],
                    start=True, stop=True, skip_group_check=True,
                )

            for _ in range(NWARM):
                warmmm()

            hlT = const.tile([H, N], bf16)
            hrT = const.tile([H, N], bf16)
            tT0 = const.tile([H, N], bf16)
            tT1 = const.tile([H, N], bf16)

            def h_piece(dst, wt, bias, s1, j0, w, on_act=True, split=False):
                # chunk 1 is shift_down (src col i-1) for xl, shift_up (i+1) for xr
                ph = ps.tile([128, w], f32, tag="ps")
                for c, s in ((0, 0), (1, s1), (2, -s1)):
                    nc.tensor.matmul(
                        ph[:, :],
                        wt[:, c, :],
                        xT[:, GD + j0 + s : GD + j0 + s + w],
                        start=(c == 0), stop=(c == 2),
                    )
                halves = ((0, w // 2), (w // 2, w)) if split else ((0, w),)
                for lo, hi in halves:
                    if on_act:
                        nc.scalar.activation(
                            dst[:, j0 + lo : j0 + hi], ph[:, lo:hi], AF.Relu,
                            bias=bias[:, 0:1], scale=1.0,
                        )
                    else:
                        nc.vector.tensor_scalar(
                            dst[:, j0 + lo : j0 + hi], ph[:, lo:hi], bias[:, 0:1],
                            0.0, ALU.add, ALU.max,
                        )

            def t_piece(j0, w):
                for wb, tT in ((wb0, tT0), (wb1, tT1)):
                    pt = ps.tile([128, w], f32, tag="ps")
                    nc.tensor.matmul(pt[:, :], wb, hlT[:, j0 : j0 + w],
                                     start=True, stop=True)
                    nc.vector.tensor_copy(tT[:, j0 : j0 + w], pt)

            _dmaq = [0]

            def out_tile(iblk, j0, w=NCH, last=False, hb=True):
                if hb:
                    heartbeat()
                ohalf = ob.tile([128, 2 * w], f32, tag="ob")
                for k, tT in ((0, tT0), (1, tT1)):
                    po = ps.tile([128, w], f32, tag="ps")
                    nc.tensor.matmul(
                        po[:, :],
                        tT[:, iblk * 128 : (iblk + 1) * 128],
                        hrT[:, j0 : j0 + w],
                        start=True, stop=True,
                    )
                    dst = ohalf[:, k : 2 * w : 2]
                    if k == 0:
                        nc.vector.tensor_scalar_add(dst, po[:, :], bb_s[:, 0:1])
                    else:
                        nc.scalar.activation(
                            dst, po[:, :], AF.Identity, bias=bb_s[:, 1:2], scale=1.0
                        )
                _dmaq[0] += 1
                eng = nc.sync if _dmaq[0] % 2 == 1 else nc.gpsimd
                dst_v = out_v[iblk][:, 2 * j0 : 2 * j0 + 2 * w]
                if last:
                    # quarter the final DMA so the tail receipt is short
                    q = w // 2
                    for qi in range(4):
                        e = nc.sync if qi % 2 == 0 else nc.gpsimd
                        e.dma_start(
                            out=dst_v[:, qi * q : (qi + 1) * q],
                            in_=ohalf[:, qi * q : (qi + 1) * q],
                        )
                else:
                    eng.dma_start(out=dst_v, in_=ohalf)

            # emission order: earliest first output DMA, PE kept dense;
            # every out_tile comes after the t/h pieces it reads; fp32
            # heartbeats hold the clock through the mid-kernel phase.
            h_piece(hlT, wlT, bl_s, -1, 0, 128, on_act=False)
            h_piece(hrT, wrT, br_s, +1, 0, NCH, split=True)
            t_piece(0, 128)
            out_tile(0, 0, 256, hb=False)
            out_tile(0, 256, 256, hb=False)
            h_piece(hlT, wlT, bl_s, -1, 128, NCH - 128, on_act=False)
            t_piece(128, NCH - 128)
            out_tile(1, 0)
            h_piece(hlT, wlT, bl_s, -1, NCH, NCH, on_act=False)
            out_tile(2, 0)
            t_piece(NCH, NCH)
            out_tile(3, 0)
            h_piece(hrT, wrT, br_s, +1, NCH, NCH)
            out_tile(4, 0)
            out_tile(0, NCH)
            out_tile(5, 0)
            out_tile(1, NCH)
            out_tile(6, 0)
            out_tile(2, NCH)
            out_tile(7, 0)
            out_tile(3, NCH)
            out_tile(4, NCH, hb=False)
            out_tile(5, NCH, hb=False)
            out_tile(6, NCH, hb=False)
            out_tile(7, NCH, last=True, hb=False)

    nc.finalize()
    return nc


def make_in_maps(x_l, W_l, b_l, W_r, b_r, W_bil, b_bil):
    import ml_dtypes

    bf16 = ml_dtypes.bfloat16

    # host-side layout: W chunks to lhsT [f=64, chunk, h], x to [64, N] with
    # zero guard columns; everything the PE touches is cast to bf16
    def w_chunks(W):
        return np.ascontiguousarray(
            np.asarray(W, np.float32).reshape(H, 3, NIN).transpose(2, 1, 0)
        ).astype(bf16)

    x_l = np.asarray(x_l, np.float32)
    xt = np.zeros((B, NIN, N + 2 * GD), np.float32)
    xt[:, :, GD : GD + N] = x_l.transpose(0, 2, 1)
    xt = xt.astype(bf16)

    com = {
        "w_lt": w_chunks(W_l),
        "b_l": np.ascontiguousarray(b_l, np.float32),
        "w_rt": w_chunks(W_r),
        "b_r": np.ascontiguousarray(b_r, np.float32),
        "w_bil": np.ascontiguousarray(np.asarray(W_bil, np.float32)).astype(bf16),
        "b_bil": np.ascontiguousarray(b_bil, np.float32),
    }
    return [{"x_t": np.ascontiguousarray(xt[c]), **com} for c in range(B)]


def kernel(x_l, W_l, b_l, W_r, b_r, W_bil, b_bil):
    from concourse import bass_utils

    if "nc" not in _cached:
        _cached["nc"] = _build()
    nc = _cached["nc"]

    in_maps = make_in_maps(x_l, W_l, b_l, W_r, b_r, W_bil, b_bil)
    res = bass_utils.run_bass_kernel_spmd(nc, in_maps, core_ids=list(range(B)))
    return np.stack([res.results[c]["out"] for c in range(B)], axis=0)
